# revision 8
# baseline (speedup 1.0000x reference)
"""AI4DEM 5^3-stencil DEM force kernel for 8 TRN2 NeuronCores.

Strategy:
  - Host: scatter particle arrays into dense 96^3 grids (one particle per
    cell), shard along Z into 8 slabs of 12 planes per core.  Each core gets
    a center slab (7 ch x 12 z x 96 x, partition = y) and one extended slab
    in DRAM (100 y_ext rows x 6 ch x 16 z_ext x 100 x_ext) whose halos wrap.
  - Device (SPMD, identical program on 8 cores): engine access patterns
    cannot start at arbitrary partitions, so the y component of each stencil
    shift is realized by DMA-staging a y-rotated copy of the extended slab
    (DMA maps DRAM rows [2-sy, 98-sy) onto partitions 0..96).  Shifts are
    grouped by sy (5 groups); within a group each shift's neighbor view is a
    plain strided view of the staged slab.  For each of the 92 shifts that
    can produce a nonzero contribution, pair forces are computed densely on
    the owned (12,96,96) region and accumulated into 6 force grids.
    The 33 remaining shifts (self + (2,2,1)/(2,2,2) offset families) can
    never produce overlap between real particles; their only effect is the
    reference's "phantom" interaction with empty cells (gathered zeros),
    which depends only on the center particle -> corrected exactly with a
    host-precomputed empty-neighbor count channel.
  - Host: gather the 9 dense output grids at the particle cells.
"""

import numpy as np

G = 96
N = 400000
NCORES = 8
ZP = G // NCORES          # 12 owned z-planes per core
ZE = ZP + 4               # 16 extended z-planes
YE = G + 4                # 100 extended y rows (DRAM only)
XE = G + 4                # 100 extended x
NCH = 7                   # x,y,z,vx,vy,vz,n_empty
OWN = ZP * G              # 1152 free elems per owned (z,x) block
FREE_C = NCH * OWN        # 8064  center slab free size
FREE_S = 6 * ZE * XE      # 9600  extended slab free size
EPS2 = 1e-8               # matches max(eps=1e-4, dist) via sqrt(dist2+eps^2)

_CACHE = {}


def _shift_sets():
    active, dropped = [], []
    for sz in range(-2, 3):
        for sy in range(-2, 3):
            for sx in range(-2, 3):
                if (sz, sy, sx) == (0, 0, 0):
                    continue
                m = sorted((abs(sz), abs(sy), abs(sx)))
                if m in ([1, 2, 2], [2, 2, 2]):
                    dropped.append((sz, sy, sx))
                else:
                    active.append((sz, sy, sx))
    assert len(active) == 92 and len(dropped) == 32
    return active, dropped


def _register_custom_ops():
    """Register the fused DVE ops we need (idempotent)."""
    import concourse.dve_ops as dve_ops_mod
    from concourse.dve_ops import DveOp, OPS, get_dve_sub_opcode, has_src1
    from concourse.dve_spec import Spec, Src0, Src1, sq, lower
    from concourse.dve_uop import DveOpSpec

    def reg(name, spec):
        for op in OPS:
            if op.name == name:
                return op
        tmp = DveOp(name, spec, subdim=False, uops_sha={})
        OPS.append(tmp)
        dve_ops_mod._SUB_OPCODE_FOR_NAME[name] = (
            dve_ops_mod._CUSTOM_DVE_ROW_BASE + len(OPS) - 1
        )
        dve_ops_mod.CUSTOM_DVE_SPECS[name] = spec
        shas = {}
        for ver in ("v3", "v4"):
            try:
                ds = DveOpSpec(
                    name=name,
                    opcode=get_dve_sub_opcode(name),
                    uops=lower(spec, ver=ver),
                    rd1_en=has_src1(spec),
                )
                shas[ver] = ds.sha(ver)
            except Exception:
                pass
        final = DveOp(name, spec, subdim=False, uops_sha=shas)
        for i, op in enumerate(OPS):
            if op.name == name:
                OPS[i] = final
                break
        return final

    sqsum = reg(
        "ANT_SQSUM2",
        Spec(
            body=sq(Src0) + sq(Src1),
            reference=lambda in0, in1, s0, s1, imm2: in0 * in0 + in1 * in1,
        ),
    )
    sqadd = reg(
        "ANT_SQADD",
        Spec(
            body=sq(Src0) + Src1,
            reference=lambda in0, in1, s0, s1, imm2: in0 * in0 + in1,
        ),
    )
    return sqsum, sqadd


def _build(d, kn, eta):
    import concourse.mybir as mybir
    from concourse.bacc import Bacc
    from concourse.tile import TileContext

    SQSUM, SQADD = _register_custom_ops()
    f32 = mybir.dt.float32
    Alu = mybir.AluOpType
    Act = mybir.ActivationFunctionType
    active, _ = _shift_sets()
    by_sy = {sy: [s for s in active if s[1] == sy] for sy in range(-2, 3)}

    nc = Bacc()
    ctr_p = nc.declare_dram_parameter("ctr", [G, FREE_C], f32, isOutput=False)
    ext_p = nc.declare_dram_parameter("ext", [YE, FREE_S], f32, isOutput=False)
    out_p = nc.declare_dram_parameter("out", [G, 9 * OWN], f32, isOutput=True)

    with TileContext(nc) as tc:
        with tc.tile_pool(name="persist", bufs=1) as pp:
            C = pp.tile([G, FREE_C], f32, tag="ctr")
            S = pp.tile([G, FREE_S], f32, tag="ext")
            OUTF = pp.tile([G, 6 * OWN], f32, tag="outf")

            for j in range(4):
                w = FREE_C // 4
                nc.sync.dma_start(C[:, j * w : (j + 1) * w], ctr_p[:, j * w : (j + 1) * w])

            def cch(i):  # flat center channel
                return C[:, i * OWN : (i + 1) * OWN]

            CV = C[:, :].rearrange("p (c z x) -> p c z x", c=NCH, z=ZP, x=G)
            SV = S[:, :].rearrange("p (c z x) -> p c z x", c=6, z=ZE, x=XE)

            def nbr(c0, c1, sz, sx):
                return SV[:, c0:c1, 2 - sz : 2 - sz + ZP, 2 - sx : 2 - sx + G]

            # ---- wall forces (channels 6..8), scoped temps, DMA out early
            with tc.tile_pool(name="wall", bufs=1) as wpool:
                W1 = wpool.tile([G, OWN], f32, tag="w1")
                W2 = wpool.tile([G, OWN], f32, tag="w2")
                WO = wpool.tile([G, 3 * OWN], f32, tag="wo")
                WC = wpool.tile([G, 2], f32, tag="wc")
                ds = G * d
                nc.vector.memset(WC[:, 0:1], kn * d)
                nc.vector.memset(WC[:, 1:2], -kn * (ds - 2.0 * d))
                for q in range(3):
                    pv = cch(q)
                    och = WO[:, q * OWN : (q + 1) * OWN]
                    nc.scalar.activation(W1[:, :], pv, Act.Relu, bias=WC[:, 0:1], scale=-kn)
                    nc.vector.scalar_tensor_tensor(
                        W2[:, :], pv, 0.0, W1[:, :], Alu.is_equal, Alu.mult
                    )
                    nc.vector.tensor_sub(och, W1[:, :], W2[:, :])
                    nc.scalar.activation(W1[:, :], pv, Act.Relu, bias=WC[:, 1:2], scale=kn)
                    nc.vector.scalar_tensor_tensor(
                        och, W1[:, :], -1.0, och, Alu.mult, Alu.add
                    )
                for j in range(3):
                    nc.sync.dma_start(
                        out_p[:, (6 + j) * OWN : (7 + j) * OWN],
                        WO[:, j * OWN : (j + 1) * OWN],
                    )

            nc.gpsimd.memset(OUTF[:, :], 0.0)

            with tc.tile_pool(name="work", bufs=1) as wp:
                D03 = wp.tile([G, 3 * OWN], f32, tag="d03")
                D36 = wp.tile([G, 3 * OWN], f32, tag="d36")
                M3 = wp.tile([G, 3 * OWN], f32, tag="m3")
                P6 = wp.tile([G, 6 * OWN], f32, tag="p6")
                S2A = wp.tile([G, OWN], f32, tag="s2a")
                DIST = wp.tile([G, OWN], f32, tag="dist")
                INV = wp.tile([G, OWN], f32, tag="inv")
                AT = wp.tile([G, OWN], f32, tag="at")
                VNA = wp.tile([G, OWN], f32, tag="vna")
                INV2 = wp.tile([G, OWN], f32, tag="inv2")
                AB = wp.tile([G, 2 * OWN], f32, tag="ab")
                CONST = wp.tile([G, 2], f32, tag="const")
                nc.vector.memset(CONST[:, 0:1], EPS2)
                nc.vector.memset(CONST[:, 1:2], 2.0 * d * kn)

                def v3(t):
                    return t[:, :].rearrange("p (c z x) -> p c z x", c=3, z=ZP, x=G)

                def chan(t, i):
                    return t[:, i * OWN : (i + 1) * OWN]

                def force_block():
                    """Consumes D03/D36, accumulates into OUTF."""
                    # dist2 = dx^2+dy^2+dz^2 (into S2A, in place for the add)
                    nc.vector._custom_dve(
                        SQSUM, out=S2A[:, :], in0=chan(D03, 0), in1=chan(D03, 1)
                    )
                    nc.vector._custom_dve(
                        SQADD, out=S2A[:, :], in0=chan(D03, 2), in1=S2A[:, :]
                    )
                    nc.scalar.activation(
                        DIST[:, :], S2A[:, :], Act.Sqrt, bias=CONST[:, 0:1]
                    )
                    nc.vector.reciprocal_approx_fast(out=INV[:, :], in_=DIST[:, :])
                    # AT = kn*relu(2d - dist)  (>0 exactly on the overlap mask)
                    nc.scalar.activation(
                        AT[:, :], DIST[:, :], Act.Relu, bias=CONST[:, 1:2], scale=-kn
                    )
                    # vn numerator
                    nc.vector.tensor_tensor(M3[:, :], D03[:, :], D36[:, :], Alu.mult)
                    nc.vector.tensor_add(VNA[:, :], chan(M3, 0), chan(M3, 1))
                    nc.vector.tensor_add(VNA[:, :], VNA[:, :], chan(M3, 2))
                    # A = -AT*inv ; B = eta*vn*mask*inv^2
                    nc.vector.scalar_tensor_tensor(
                        AB[:, 0:OWN], AT[:, :], -1.0, INV[:, :], Alu.mult, Alu.mult
                    )
                    nc.vector.tensor_tensor(
                        INV2[:, :], INV[:, :], INV[:, :], Alu.mult
                    )
                    nc.vector.scalar_tensor_tensor(
                        INV2[:, :], AT[:, :], 0.0, INV2[:, :], Alu.is_gt, Alu.mult
                    )
                    nc.vector.scalar_tensor_tensor(
                        AB[:, OWN : 2 * OWN],
                        VNA[:, :],
                        eta,
                        INV2[:, :],
                        Alu.mult,
                        Alu.mult,
                    )

                def products_and_acc():
                    for h in range(2):
                        a = AB[:, h * OWN : (h + 1) * OWN]
                        for q in range(3):
                            nc.vector.tensor_tensor(
                                chan(P6, 3 * h + q), a, chan(D03, q), Alu.mult
                            )
                    nc.gpsimd.tensor_add(OUTF[:, :], OUTF[:, :], P6[:, :])

                for sy in (-2, -1, 0, 1, 2):
                    # stage the y-rotated extended slab: S[p] = grid[y = p - sy]
                    for j in range(8):
                        w = FREE_S // 8
                        nc.sync.dma_start(
                            S[:, j * w : (j + 1) * w],
                            ext_p[2 - sy : 2 - sy + G, j * w : (j + 1) * w],
                        )
                    for sz, _sy, sx in by_sy[sy]:
                        nc.vector.tensor_tensor(
                            v3(D03), CV[:, 0:3], nbr(0, 3, sz, sx), Alu.subtract
                        )
                        nc.gpsimd.tensor_tensor(
                            v3(D36), CV[:, 3:6], nbr(3, 6, sz, sx), Alu.subtract
                        )
                        force_block()
                        products_and_acc()

                # phantom correction for the 32 dropped shifts
                nc.vector.tensor_copy(D03[:, :], C[:, 0 : 3 * OWN])
                nc.gpsimd.tensor_copy(D36[:, :], C[:, 3 * OWN : 6 * OWN])
                force_block()
                nemv = cch(6)
                nc.vector.tensor_tensor(AB[:, 0:OWN], AB[:, 0:OWN], nemv, Alu.mult)
                nc.vector.tensor_tensor(
                    AB[:, OWN : 2 * OWN], AB[:, OWN : 2 * OWN], nemv, Alu.mult
                )
                products_and_acc()

                for j in range(6):
                    nc.sync.dma_start(
                        out_p[:, j * OWN : (j + 1) * OWN],
                        OUTF[:, j * OWN : (j + 1) * OWN],
                    )

    nc.finalize()
    return nc


def _host_prep(inputs):
    d = float(np.asarray(inputs["d"]))
    x = np.asarray(inputs["compressed_x_grid"], np.float32)
    y = np.asarray(inputs["compressed_y_grid"], np.float32)
    z = np.asarray(inputs["compressed_z_grid"], np.float32)
    vx = np.asarray(inputs["compressed_vx_grid"], np.float32)
    vy = np.asarray(inputs["compressed_vy_grid"], np.float32)
    vz = np.asarray(inputs["compressed_vz_grid"], np.float32)

    cx = np.round(x / np.float32(d)).astype(np.int32)
    cy = np.round(y / np.float32(d)).astype(np.int32)
    cz = np.round(z / np.float32(d)).astype(np.int32)

    grids = np.zeros((NCH, G, G, G), np.float32)
    for i, v in enumerate((x, y, z, vx, vy, vz)):
        grids[i, cz, cy, cx] = v
    occ = np.zeros((G, G, G), np.float32)
    occ[cz, cy, cx] = 1.0

    _, dropped = _shift_sets()
    nocc = np.zeros((G, G, G), np.float32)
    for s in dropped:
        nocc += np.roll(occ, s, axis=(0, 1, 2))
    grids[6] = np.float32(len(dropped)) - nocc

    ys = np.arange(-2, G + 2) % G
    xs = np.arange(-2, G + 2) % G
    in_maps = []
    for k in range(NCORES):
        z0 = k * ZP
        # center slab: (y, ch, z_owned, x_owned)
        ctr = np.ascontiguousarray(
            grids[:, z0 : z0 + ZP].transpose(2, 0, 1, 3)
        ).reshape(G, FREE_C)
        # extended slab: (y_ext, ch, z_ext, x_ext), 6 data channels
        zs = np.arange(z0 - 2, z0 + ZP + 2) % G
        ext = grids[0:6, zs][:, :, ys][:, :, :, xs]  # (6,16,100,100)
        ext = np.ascontiguousarray(ext.transpose(2, 0, 1, 3)).reshape(YE, FREE_S)
        in_maps.append({"ctr": ctr, "ext": ext})
    return in_maps, (cz, cy, cx)


def kernel(**inputs):
    from concourse.bass_utils import run_bass_kernel_spmd

    d = float(np.asarray(inputs["d"]))
    kn = float(np.asarray(inputs["kn"]))
    eta = float(np.asarray(inputs["damping_coefficient_Eta"]))

    in_maps, (cz, cy, cx) = _host_prep(inputs)

    key = (d, kn, eta)
    if key not in _CACHE:
        _CACHE[key] = _build(d, kn, eta)
    nc = _CACHE[key]

    res = run_bass_kernel_spmd(nc, in_maps, core_ids=list(range(NCORES)))
    full = np.empty((9, G, G, G), np.float32)
    for k in range(NCORES):
        o = np.asarray(res.results[k]["out"], np.float32).reshape(G, 9, ZP, G)
        full[:, k * ZP : (k + 1) * ZP] = o.transpose(1, 2, 0, 3)
    return full[:, cz, cy, cx]


# revision 9
# speedup vs baseline: 1.4410x; 1.4410x over previous
"""AI4DEM 5^3-stencil DEM force kernel for 8 TRN2 NeuronCores.

Strategy:
  - Host: scatter particle arrays into dense 96^3 grids (one particle per
    cell), shard along Z into 8 slabs of 12 planes per core.  Each core gets
    a center slab (7 ch x 12 z x 96 x, partition = y) and one extended slab
    in DRAM (100 y_ext rows x 6 ch x 16 z_ext x 100 x_ext) whose halos wrap.
  - Device (SPMD, identical program on 8 cores): engine access patterns
    cannot start at arbitrary partitions, so the y component of each stencil
    shift is realized by DMA-staging a y-rotated copy of the extended slab
    (DMA maps DRAM rows [2-sy, 98-sy) onto partitions 0..96).  Shifts are
    grouped by sy (5 groups); within a group each shift's neighbor view is a
    plain strided view of the staged slab.  For each of the 92 shifts that
    can produce a nonzero contribution, pair forces are computed densely on
    the owned (12,96,96) region and accumulated into 6 force grids.
    The 33 remaining shifts (self + (2,2,1)/(2,2,2) offset families) can
    never produce overlap between real particles; their only effect is the
    reference's "phantom" interaction with empty cells (gathered zeros),
    which depends only on the center particle -> corrected exactly with a
    host-precomputed empty-neighbor count channel.
  - Host: gather the 9 dense output grids at the particle cells.
"""

import numpy as np

G = 96
N = 400000
NCORES = 8
ZP = G // NCORES          # 12 owned z-planes per core
ZE = ZP + 4               # 16 extended z-planes
YE = G + 4                # 100 extended y rows (DRAM only)
XE = G + 4                # 100 extended x
NCH = 7                   # x,y,z,vx,vy,vz,n_empty
OWN = ZP * G              # 1152 free elems per owned (z,x) block
FREE_C = NCH * OWN        # 8064  center slab free size
FREE_S = 6 * ZE * XE      # 9600  extended slab free size
EPS2 = 1e-8               # matches max(eps=1e-4, dist) via sqrt(dist2+eps^2)

_CACHE = {}


def _shift_sets():
    active, dropped = [], []
    for sz in range(-2, 3):
        for sy in range(-2, 3):
            for sx in range(-2, 3):
                if (sz, sy, sx) == (0, 0, 0):
                    continue
                m = sorted((abs(sz), abs(sy), abs(sx)))
                if m in ([1, 2, 2], [2, 2, 2]):
                    dropped.append((sz, sy, sx))
                else:
                    active.append((sz, sy, sx))
    assert len(active) == 92 and len(dropped) == 32
    return active, dropped


def _register_custom_ops():
    """Register the fused DVE ops we need (idempotent)."""
    import concourse.dve_ops as dve_ops_mod
    from concourse.dve_ops import DveOp, OPS, get_dve_sub_opcode, has_src1
    from concourse.dve_spec import Spec, Src0, Src1, sq, lower
    from concourse.dve_uop import DveOpSpec

    def reg(name, spec):
        for op in OPS:
            if op.name == name:
                return op
        tmp = DveOp(name, spec, subdim=False, uops_sha={})
        OPS.append(tmp)
        dve_ops_mod._SUB_OPCODE_FOR_NAME[name] = (
            dve_ops_mod._CUSTOM_DVE_ROW_BASE + len(OPS) - 1
        )
        dve_ops_mod.CUSTOM_DVE_SPECS[name] = spec
        shas = {}
        for ver in ("v3", "v4"):
            try:
                ds = DveOpSpec(
                    name=name,
                    opcode=get_dve_sub_opcode(name),
                    uops=lower(spec, ver=ver),
                    rd1_en=has_src1(spec),
                )
                shas[ver] = ds.sha(ver)
            except Exception:
                pass
        final = DveOp(name, spec, subdim=False, uops_sha=shas)
        for i, op in enumerate(OPS):
            if op.name == name:
                OPS[i] = final
                break
        return final

    sqsum = reg(
        "ANT_SQSUM2",
        Spec(
            body=sq(Src0) + sq(Src1),
            reference=lambda in0, in1, s0, s1, imm2: in0 * in0 + in1 * in1,
        ),
    )
    sqadd = reg(
        "ANT_SQADD",
        Spec(
            body=sq(Src0) + Src1,
            reference=lambda in0, in1, s0, s1, imm2: in0 * in0 + in1,
        ),
    )
    return sqsum, sqadd


def _build(d, kn, eta):
    import concourse.mybir as mybir
    from concourse.bacc import Bacc
    from concourse.tile import TileContext

    SQSUM, SQADD = _register_custom_ops()
    f32 = mybir.dt.float32
    Alu = mybir.AluOpType
    Act = mybir.ActivationFunctionType
    active, _ = _shift_sets()
    by_sy = {sy: [s for s in active if s[1] == sy] for sy in range(-2, 3)}

    nc = Bacc()
    ctr_p = nc.declare_dram_parameter("ctr", [G, FREE_C], f32, isOutput=False)
    ext_p = nc.declare_dram_parameter("ext", [YE, FREE_S], f32, isOutput=False)
    out_p = nc.declare_dram_parameter("out", [G, 9 * OWN], f32, isOutput=True)
    eye_p = nc.declare_dram_parameter("eye", [G, G], f32, isOutput=False)

    with TileContext(nc) as tc:
        with tc.tile_pool(name="persist", bufs=1) as pp:
            C = pp.tile([G, FREE_C], f32, tag="ctr")
            S = pp.tile([G, FREE_S], f32, tag="ext")
            OUTF = pp.tile([G, 6 * OWN], f32, tag="outf")

            for j in range(4):
                w = FREE_C // 4
                nc.sync.dma_start(C[:, j * w : (j + 1) * w], ctr_p[:, j * w : (j + 1) * w])

            def cch(i):  # flat center channel
                return C[:, i * OWN : (i + 1) * OWN]

            CV = C[:, :].rearrange("p (c z x) -> p c z x", c=NCH, z=ZP, x=G)
            SV = S[:, :].rearrange("p (c z x) -> p c z x", c=6, z=ZE, x=XE)

            def nbr(c0, c1, sz, sx):
                return SV[:, c0:c1, 2 - sz : 2 - sz + ZP, 2 - sx : 2 - sx + G]

            # ---- wall forces (channels 6..8), scoped temps, DMA out early
            with tc.tile_pool(name="wall", bufs=1) as wpool:
                W1 = wpool.tile([G, OWN], f32, tag="w1")
                W2 = wpool.tile([G, OWN], f32, tag="w2")
                WO = wpool.tile([G, 3 * OWN], f32, tag="wo")
                WC = wpool.tile([G, 2], f32, tag="wc")
                ds = G * d
                nc.vector.memset(WC[:, 0:1], kn * d)
                nc.vector.memset(WC[:, 1:2], -kn * (ds - 2.0 * d))
                for q in range(3):
                    pv = cch(q)
                    och = WO[:, q * OWN : (q + 1) * OWN]
                    nc.scalar.activation(W1[:, :], pv, Act.Relu, bias=WC[:, 0:1], scale=-kn)
                    nc.vector.scalar_tensor_tensor(
                        W2[:, :], pv, 0.0, W1[:, :], Alu.is_equal, Alu.mult
                    )
                    nc.vector.tensor_sub(och, W1[:, :], W2[:, :])
                    nc.scalar.activation(W1[:, :], pv, Act.Relu, bias=WC[:, 1:2], scale=kn)
                    nc.vector.scalar_tensor_tensor(
                        och, W1[:, :], -1.0, och, Alu.mult, Alu.add
                    )
                for j in range(3):
                    nc.sync.dma_start(
                        out_p[:, (6 + j) * OWN : (7 + j) * OWN],
                        WO[:, j * OWN : (j + 1) * OWN],
                    )

            nc.gpsimd.memset(OUTF[:, :], 0.0)

            with (
                tc.tile_pool(name="work", bufs=1) as wp,
                tc.tile_pool(name="psum", bufs=1, space="PSUM") as psp,
            ):
                PSA = psp.tile([G, 3 * OWN], f32, tag="psa")
                EYE = wp.tile([G, G], f32, tag="eye")
                nc.sync.dma_start(EYE[:, :], eye_p[:, :])
                D03 = wp.tile([G, 3 * OWN], f32, tag="d03")
                D36 = wp.tile([G, 3 * OWN], f32, tag="d36")
                M3 = wp.tile([G, 3 * OWN], f32, tag="m3")
                P6 = wp.tile([G, 6 * OWN], f32, tag="p6")
                S2A = wp.tile([G, OWN], f32, tag="s2a")
                DIST = wp.tile([G, OWN], f32, tag="dist")
                INV = wp.tile([G, OWN], f32, tag="inv")
                AT = wp.tile([G, OWN], f32, tag="at")
                VNA = wp.tile([G, OWN], f32, tag="vna")
                INV2 = wp.tile([G, OWN], f32, tag="inv2")
                AB = wp.tile([G, 2 * OWN], f32, tag="ab")
                CONST = wp.tile([G, 2], f32, tag="const")
                nc.vector.memset(CONST[:, 0:1], EPS2)
                nc.vector.memset(CONST[:, 1:2], 2.0 * d * kn)

                def v3(t):
                    return t[:, :].rearrange("p (c z x) -> p c z x", c=3, z=ZP, x=G)

                def chan(t, i):
                    return t[:, i * OWN : (i + 1) * OWN]

                def force_block():
                    """Consumes D03/D36, accumulates into OUTF."""
                    # dist2 = dx^2+dy^2+dz^2 (into S2A, in place for the adds).
                    # sq(Src1) in a custom DVE op hits a ~14x slow uop path, so
                    # the first square runs on the idle ScalarE instead.
                    nc.scalar.activation(S2A[:, :], chan(D03, 0), Act.Square)
                    nc.vector._custom_dve(
                        SQADD, out=S2A[:, :], in0=chan(D03, 1), in1=S2A[:, :]
                    )
                    nc.vector._custom_dve(
                        SQADD, out=S2A[:, :], in0=chan(D03, 2), in1=S2A[:, :]
                    )
                    nc.scalar.activation(
                        DIST[:, :], S2A[:, :], Act.Sqrt, bias=CONST[:, 0:1]
                    )
                    nc.vector.reciprocal_approx_fast(out=INV[:, :], in_=DIST[:, :])
                    # AT = kn*relu(2d - dist)  (>0 exactly on the overlap mask)
                    nc.scalar.activation(
                        AT[:, :], DIST[:, :], Act.Relu, bias=CONST[:, 1:2], scale=-kn
                    )
                    # vn numerator
                    nc.vector.tensor_tensor(M3[:, :], D03[:, :], D36[:, :], Alu.mult)
                    nc.vector.tensor_add(VNA[:, :], chan(M3, 0), chan(M3, 1))
                    nc.vector.tensor_add(VNA[:, :], VNA[:, :], chan(M3, 2))
                    # A = -AT*inv ; B = eta*vn*mask*inv^2
                    nc.vector.scalar_tensor_tensor(
                        AB[:, 0:OWN], AT[:, :], -1.0, INV[:, :], Alu.mult, Alu.mult
                    )
                    nc.scalar.activation(INV2[:, :], INV[:, :], Act.Square)
                    nc.vector.scalar_tensor_tensor(
                        INV2[:, :], AT[:, :], 0.0, INV2[:, :], Alu.is_gt, Alu.mult
                    )
                    nc.vector.scalar_tensor_tensor(
                        AB[:, OWN : 2 * OWN],
                        VNA[:, :],
                        eta,
                        INV2[:, :],
                        Alu.mult,
                        Alu.mult,
                    )

                def products_and_acc(first, last):
                    for h in range(2):
                        a = AB[:, h * OWN : (h + 1) * OWN]
                        for q in range(3):
                            nc.vector.tensor_tensor(
                                chan(P6, 3 * h + q), a, chan(D03, q), Alu.mult
                            )
                    half = 3 * OWN
                    for j in range(7):
                        w0 = j * 512
                        w1 = min(half, w0 + 512)
                        nc.tensor.matmul(
                            PSA[:, w0:w1],
                            EYE[:, :],
                            P6[:, w0:w1],
                            start=first,
                            stop=last,
                        )
                    nc.gpsimd.tensor_add(
                        OUTF[:, half:], OUTF[:, half:], P6[:, half:]
                    )

                first = True
                for sy in (-2, -1, 0, 1, 2):
                    # stage the y-rotated extended slab: S[p] = grid[y = p - sy]
                    for j in range(8):
                        w = FREE_S // 8
                        nc.sync.dma_start(
                            S[:, j * w : (j + 1) * w],
                            ext_p[2 - sy : 2 - sy + G, j * w : (j + 1) * w],
                        )
                    for sz, _sy, sx in by_sy[sy]:
                        nc.vector.tensor_tensor(
                            v3(D03), CV[:, 0:3], nbr(0, 3, sz, sx), Alu.subtract
                        )
                        nc.gpsimd.tensor_tensor(
                            v3(D36), CV[:, 3:6], nbr(3, 6, sz, sx), Alu.subtract
                        )
                        force_block()
                        products_and_acc(first, False)
                        first = False

                # phantom correction for the 32 dropped shifts
                nc.vector.tensor_copy(D03[:, :], C[:, 0 : 3 * OWN])
                nc.gpsimd.tensor_copy(D36[:, :], C[:, 3 * OWN : 6 * OWN])
                force_block()
                nemv = cch(6)
                nc.vector.tensor_tensor(AB[:, 0:OWN], AB[:, 0:OWN], nemv, Alu.mult)
                nc.vector.tensor_tensor(
                    AB[:, OWN : 2 * OWN], AB[:, OWN : 2 * OWN], nemv, Alu.mult
                )
                products_and_acc(False, True)
                nc.vector.tensor_copy(OUTF[:, 0 : 3 * OWN], PSA[:, :])

                for j in range(6):
                    nc.sync.dma_start(
                        out_p[:, j * OWN : (j + 1) * OWN],
                        OUTF[:, j * OWN : (j + 1) * OWN],
                    )

    nc.finalize()
    return nc


def _host_prep(inputs):
    d = float(np.asarray(inputs["d"]))
    x = np.asarray(inputs["compressed_x_grid"], np.float32)
    y = np.asarray(inputs["compressed_y_grid"], np.float32)
    z = np.asarray(inputs["compressed_z_grid"], np.float32)
    vx = np.asarray(inputs["compressed_vx_grid"], np.float32)
    vy = np.asarray(inputs["compressed_vy_grid"], np.float32)
    vz = np.asarray(inputs["compressed_vz_grid"], np.float32)

    cx = np.round(x / np.float32(d)).astype(np.int32)
    cy = np.round(y / np.float32(d)).astype(np.int32)
    cz = np.round(z / np.float32(d)).astype(np.int32)

    grids = np.zeros((NCH, G, G, G), np.float32)
    for i, v in enumerate((x, y, z, vx, vy, vz)):
        grids[i, cz, cy, cx] = v
    occ = np.zeros((G, G, G), np.float32)
    occ[cz, cy, cx] = 1.0

    _, dropped = _shift_sets()
    nocc = np.zeros((G, G, G), np.float32)
    for s in dropped:
        nocc += np.roll(occ, s, axis=(0, 1, 2))
    grids[6] = np.float32(len(dropped)) - nocc

    ys = np.arange(-2, G + 2) % G
    xs = np.arange(-2, G + 2) % G
    in_maps = []
    for k in range(NCORES):
        z0 = k * ZP
        # center slab: (y, ch, z_owned, x_owned)
        ctr = np.ascontiguousarray(
            grids[:, z0 : z0 + ZP].transpose(2, 0, 1, 3)
        ).reshape(G, FREE_C)
        # extended slab: (y_ext, ch, z_ext, x_ext), 6 data channels
        zs = np.arange(z0 - 2, z0 + ZP + 2) % G
        ext = grids[0:6, zs][:, :, ys][:, :, :, xs]  # (6,16,100,100)
        ext = np.ascontiguousarray(ext.transpose(2, 0, 1, 3)).reshape(YE, FREE_S)
        in_maps.append({"ctr": ctr, "ext": ext, "eye": np.eye(G, dtype=np.float32)})
    return in_maps, (cz, cy, cx)


def kernel(**inputs):
    from concourse.bass_utils import run_bass_kernel_spmd

    d = float(np.asarray(inputs["d"]))
    kn = float(np.asarray(inputs["kn"]))
    eta = float(np.asarray(inputs["damping_coefficient_Eta"]))

    in_maps, (cz, cy, cx) = _host_prep(inputs)

    key = (d, kn, eta)
    if key not in _CACHE:
        _CACHE[key] = _build(d, kn, eta)
    nc = _CACHE[key]

    res = run_bass_kernel_spmd(nc, in_maps, core_ids=list(range(NCORES)))
    full = np.empty((9, G, G, G), np.float32)
    for k in range(NCORES):
        o = np.asarray(res.results[k]["out"], np.float32).reshape(G, 9, ZP, G)
        full[:, k * ZP : (k + 1) * ZP] = o.transpose(1, 2, 0, 3)
    return full[:, cz, cy, cx]
